# revision 29
# baseline (speedup 1.0000x reference)
"""Trainium2 Bass kernel for nn_AttnResBlock (RMSNorm -> scalar proj ->
softmax over depth N -> weighted sum of history).

Reference computation (per (b, t) position, D=1024, N=13):
  ms      = mean_d(V^2)
  logits  = rsqrt(ms + eps) * sum_d(V * (rms_weight * w_proj))
  alpha   = softmax_n(logits)
  out     = sum_n alpha_n * V_n

Sharding: B*T = 4096 positions split contiguously across 8 cores (512
positions each). All reductions are over D and N, both core-local -> no
collectives. Tiny [D] params are replicated (pre-broadcast on host).

V is shipped bf16 (halves HBM traffic; ~7e-3 rel err, well inside the
2e-2 gate). Per-core layout: partitions = positions (tiles of 128), free
dim = (n, d); host pre-transposes V to [pos, n, d] so per-partition DMA
runs are contiguous.

Engine split per tile, from measured HW op costs (DVE STT 1.22us/slice,
ACT Square+accum 1.43us, Pool TT ~2ns/elem, no 16-bit speedup anywhere):
  DVE : 13 w_proj dots (fused STT mul+accum) + 1 square + softmax smalls
  ACT : 12 squares (Square+accum), rsqrt via ln/exp, softmax exp,
        evacuation of PSUM half 0
  Pool: diag-alpha bank build (chunked so PE starts early), logits mul
  PE  : weighted sum over N as 2x13 accumulating bf16 matmuls with the
        diagonal-alpha stationary, half-D PSUM chains so evacuation of
        half 0 overlaps the half-1 chain
  SP/gpsimd: DMA rings (one queue tops out ~227 GB/s, so loads are split)
"""

import numpy as np

import concourse.bacc as bacc
import concourse.hw_specs as hw_specs
import concourse.mybir as mybir
from concourse.bass import ts
from concourse.bass_utils import run_bass_kernel_spmd
from concourse.tile import TileContext

N, B, T, D = 13, 2, 2048, 1024
N_CORES = 8
POS_TOTAL = B * T                    # 4096
POS_PER_CORE = POS_TOTAL // N_CORES  # 512
P = 128                              # SBUF partitions = positions per tile
TILES = POS_PER_CORE // P            # 4
EPS = float(np.finfo(np.float32).eps)

F32 = mybir.dt.float32
BF16 = mybir.dt.bfloat16
Alu = mybir.AluOpType
Act = mybir.ActivationFunctionType

SQ_DVE = set()       # squares done on DVE instead of ACT (balance knob)
# load groups: (n_lo, n_hi, ring); ring 0 = SP HWDGE, ring 1 = gpsimd SWDGE
# tile 0 uses fine groups so the first reduce-slices start ~3us in
LOAD_GROUPS_FIRST = [
    (0, 1, 0), (1, 2, 1), (2, 4, 0), (4, 6, 1),
    (6, 8, 0), (8, 10, 1), (10, 13, 0),
]
LOAD_GROUPS = [(0, 4, 0), (4, 7, 1), (7, 10, 0), (10, 13, 1)]
DGB_CHUNKS = [(0, 5), (5, 10), (10, 13)]
MM_FREE = 512  # free dim per matmul; PSUM bank limit for 4B

_CACHE = {}

_ACT_SET = "natural_log_exp_and_others"  # contains ln, exp, square, copy


def _patched_tables(orig):
    def fn(module_arch):
        t = orig(module_arch)
        return {k: (v if k == _ACT_SET else set()) for k, v in t.items()}

    return fn


def _build():
    nc = bacc.Bacc(None, target_bir_lowering=False)
    v = nc.dram_tensor("v", [POS_PER_CORE, N, D], BF16, kind="ExternalInput")
    wcb = nc.dram_tensor("wcb", [P, D], BF16, kind="ExternalInput")
    identb = nc.dram_tensor("identb", [P, N, P], BF16, kind="ExternalInput")
    o = nc.dram_tensor("o", [POS_PER_CORE, D], F32, kind="ExternalOutput")

    with TileContext(nc) as tc:
        with (
            tc.tile_pool(name="cst", bufs=1) as cst,
            tc.tile_pool(name="vp", bufs=4) as vp,
            tc.tile_pool(name="sm", bufs=3) as sm,
            tc.tile_pool(name="dg", bufs=3) as dg,
            tc.tile_pool(name="ob", bufs=2) as ob,
            tc.tile_pool(name="ps", bufs=3, space="PSUM") as psp,
        ):
            wct = cst.tile([P, D], BF16)
            idb = cst.tile([P, N, P], BF16)
            epst = cst.tile([P, 1], F32)
            trash_dve = cst.tile([P, D], BF16)
            trash_act = cst.tile([P, D], BF16)
            nc.sync.dma_start(out=wct[:], in_=wcb[:, :])
            nc.vector.memset(epst[:], EPS)

            rings = [nc.sync, nc.gpsimd]
            for t in range(TILES):
                # ---- load [128 pos, 13 n, 1024 d] bf16, split across rings ----
                vt = vp.tile([P, N, D], BF16, tag="vt")
                groups = LOAD_GROUPS_FIRST if t == 0 else LOAD_GROUPS
                for lo, hi, r in groups:
                    rings[r].dma_start(out=vt[:, lo:hi, :], in_=v[ts(t, P), lo:hi, :])
                if t == 0:
                    # idb is not needed until the first dgb build; issuing it
                    # after tile 0's loads keeps the queues clear for ramp
                    nc.gpsimd.dma_start(out=idb[:], in_=identb[:, :, :])

                dotv = sm.tile([P, N], F32, tag="dotv")
                msv = sm.tile([P, N], F32, tag="msv")
                for n in range(N):
                    nc.vector.scalar_tensor_tensor(
                        out=trash_dve[:],
                        in0=vt[:, n, :],
                        scalar=0.0,
                        in1=wct[:],
                        op0=Alu.bypass,
                        op1=Alu.mult,
                        accum_out=dotv[:, n : n + 1],
                    )
                    # squares walk n two slices ahead of the dots so DVE and
                    # ACT stream different vt regions (less SBUF contention)
                    m = (n + 2) % N
                    nc.scalar.activation(
                        out=trash_act[:],
                        in_=vt[:, m, :],
                        func=Act.Square,
                        accum_out=msv[:, m : m + 1],
                    )

                # ---- softmax over n (high priority: unblocks PE + reuse) ----
                with tc.high_priority(offset=100):
                    # rsqrt(mean + eps) = exp(-0.5 * ln(ms/D + eps))
                    lnv = sm.tile([P, N], F32, tag="lnv")
                    rsq = sm.tile([P, N], F32, tag="rsq")
                    nc.scalar.activation(
                        lnv[:], msv[:], Act.Ln, bias=epst[:], scale=1.0 / D
                    )
                    nc.scalar.activation(rsq[:], lnv[:], Act.Exp, scale=-0.5)
                    lg = sm.tile([P, N], F32, tag="lg")
                    nc.vector.tensor_tensor(lg[:], dotv[:], rsq[:], Alu.mult)
                    negm = sm.tile([P, 1], F32, tag="negm")
                    nc.vector.tensor_reduce(
                        out=negm[:], in_=lg[:], op=Alu.max,
                        axis=mybir.AxisListType.X, negate=True,
                    )
                    # exp with fused denominator accumulation
                    ev = sm.tile([P, N], F32, tag="ev")
                    ssum = sm.tile([P, 1], F32, tag="ssum")
                    nc.scalar.activation(
                        ev[:], lg[:], Act.Exp, bias=negm[:], accum_out=ssum[:]
                    )
                    rcp = sm.tile([P, 1], F32, tag="rcp")
                    nc.vector.reciprocal(rcp[:], ssum[:])
                    alpha = sm.tile([P, N, 1], F32, tag="alpha")
                    nc.vector.tensor_scalar(
                        alpha[:, :, 0], ev[:], rcp[:], None, Alu.mult
                    )

                    # ---- weighted sum over n: PE with diagonal-alpha ----
                    # dgb built in chunks on Pool so PE can start after the
                    # first chunk
                    dgb = dg.tile([P, N, P], BF16, tag="dgb")
                    for lo, hi in DGB_CHUNKS:
                        nc.gpsimd.tensor_tensor(
                            dgb[:, lo:hi, :],
                            idb[:, lo:hi, :],
                            alpha[:, lo:hi].broadcast_to((P, hi - lo, P)),
                            Alu.mult,
                        )
                    ps = psp.tile([P, D], F32, tag="ps")
                    osb = ob.tile([P, D], F32, tag="osb")
                    for h in range(D // MM_FREE):
                        for n in range(N):
                            nc.tensor.matmul(
                                ps[:, ts(h, MM_FREE)],
                                dgb[:, n, :],
                                vt[:, n, ts(h, MM_FREE)],
                                start=(n == 0),
                                stop=(n == N - 1),
                            )
                        # evacuate this half while the next half's chain
                        # runs; halves go to different engines so the last
                        # tile's evacuations overlap
                        if h == 0:
                            nc.scalar.copy(
                                osb[:, ts(h, MM_FREE)], ps[:, ts(h, MM_FREE)]
                            )
                        else:
                            nc.vector.tensor_scalar(
                                osb[:, ts(h, MM_FREE)], ps[:, ts(h, MM_FREE)],
                                0.0, None, Alu.bypass,
                            )
                        if t == TILES - 1 and h == 1:
                            # split the very last store so the final transfer
                            # (which gates the end-of-kernel barrier) is short
                            nc.sync.dma_start(
                                out=o[ts(t, P), 512:768], in_=osb[:, 512:768]
                            )
                            nc.sync.dma_start(
                                out=o[ts(t, P), 768:1024], in_=osb[:, 768:1024]
                            )
                        else:
                            nc.sync.dma_start(
                                out=o[ts(t, P), ts(h, MM_FREE)],
                                in_=osb[:, ts(h, MM_FREE)],
                            )

    orig = hw_specs.get_activation_tables
    bacc_orig = bacc.get_activation_tables
    try:
        hw_specs.get_activation_tables = _patched_tables(orig)
        bacc.get_activation_tables = hw_specs.get_activation_tables
        nc.finalize()
    finally:
        hw_specs.get_activation_tables = orig
        bacc.get_activation_tables = bacc_orig
    return nc


def _host_prep(V, rms_weight, w_proj):
    import ml_dtypes

    wc = (rms_weight.astype(np.float32) * w_proj.astype(np.float32)).astype(
        ml_dtypes.bfloat16
    )
    wcb = np.ascontiguousarray(np.broadcast_to(wc, (P, D)))
    identb = np.ascontiguousarray(
        np.broadcast_to(
            np.eye(P, dtype=ml_dtypes.bfloat16), (N, P, P)
        ).transpose(1, 0, 2)
    )
    # [N, B*T, D] -> [B*T, N, D] so per-partition DMA runs are contiguous
    vt = np.ascontiguousarray(
        V.reshape(N, POS_TOTAL, D).transpose(1, 0, 2).astype(ml_dtypes.bfloat16)
    )
    in_maps = []
    for c in range(N_CORES):
        shard = vt[c * POS_PER_CORE : (c + 1) * POS_PER_CORE]
        in_maps.append({"v": shard, "wcb": wcb, "identb": identb})
    return in_maps


def kernel(V, rms_weight, w_proj):
    if "nc" not in _CACHE:
        _CACHE["nc"] = _build()
    nc = _CACHE["nc"]
    in_maps = _host_prep(
        np.asarray(V), np.asarray(rms_weight), np.asarray(w_proj)
    )
    res = run_bass_kernel_spmd(nc, in_maps, core_ids=list(range(N_CORES)), trace=False)
    out = np.concatenate([res.results[c]["o"] for c in range(N_CORES)], axis=0)
    return out.reshape(B, T, D)
